# revision 75
# baseline (speedup 1.0000x reference)
"""Causal self-attention (B=2, T=2048, C=1024, H=16) on 8 trn2 NeuronCores.

Sharding: tensor-parallel over heads. Each core owns 2 heads (128 channels):
  - qkv projection for its 128 q/k/v columns (full x, transposed layout xT)
  - causal flash-style attention for its 2 heads x 2 batches
  - output projection rows for its 128 channels -> partial (4096, 1024) output
Host sums the 8 partial outputs (the "all-reduce") and adds bproj once.

The datapath is bf16 (inputs converted on host): halves HBM traffic for x and
the weights, runs the PE at full rate at every matmul width (fp32r pays 4x
below 256 cols), and unlocks the DVE 2x mode for the causal-mask multiplies.
PSUM accumulation stays fp32; the projection PSUM is DMAed straight to DRAM
(fp32 partial output, bias added on host), so no output-side elementwise ops.
Softmax skips max-subtraction (logits ~ N(0,1); exp is safe).
q/k/v/y live in per-512-token-chunk tiles so the qkv / attention / projection
phases pipeline instead of serializing on whole-tensor dependencies.
Dependent stores (y repack, projection output) issue from the gpsimd SWDGE
queue so the sync-engine DMA queue (x/weight prefetch) never head-of-line
blocks on compute.
"""

import sys

if "/opt/trn_rl_repo" not in sys.path:
    sys.path.insert(0, "/opt/trn_rl_repo")

from collections import deque

import numpy as np

import concourse.bass as bass
import concourse.mybir as mybir
import concourse.tile as tile
from concourse import bacc
from concourse.bass_utils import run_bass_kernel_spmd

# Problem shape (hardcoded per contest contract)
B, T, C, H = 2, 2048, 1024, 16
D = C // H                # 64 head dim
N_CORES = 8
HPC = H // N_CORES        # 2 heads per core
CH = HPC * D              # 128 channels per core
TALL = B * T              # 4096 flattened tokens
NCT = C // 128            # 8 contraction tiles
TCH = 512                 # t-chunk
NCHUNK = TALL // TCH      # 8
NQC = T // 512            # 4 q-chunks per batch
NKT_B = T // 128          # 16 k-tiles per batch
F32 = mybir.dt.float32
BF16 = mybir.dt.bfloat16


def build_nc(phases=("qkv", "att", "proj"), repeat=1, cfg=None):
    cfg = cfg or {}
    ST_W = cfg.get("st_w", 1024)
    ST_B = cfg.get("st_bufs", 2)
    MM_B = cfg.get("mm_bufs", 2)
    YT_B = cfg.get("yt_bufs", 2)
    nc = bacc.Bacc("TRN2", target_bir_lowering=False, debug=False)

    xT = nc.dram_tensor("xT", (NCT, 128, TALL), BF16, kind="ExternalInput").ap()
    wq = nc.dram_tensor("wq", (NCT, 128, CH), BF16, kind="ExternalInput").ap()
    wk = nc.dram_tensor("wk", (NCT, 128, CH), BF16, kind="ExternalInput").ap()
    wv = nc.dram_tensor("wv", (NCT, 128, CH), BF16, kind="ExternalInput").ap()
    bq = nc.dram_tensor("bq", (CH, 1), F32, kind="ExternalInput").ap()
    bk = nc.dram_tensor("bk", (CH, 1), F32, kind="ExternalInput").ap()
    bvr = nc.dram_tensor("bvr", (1, CH), BF16, kind="ExternalInput").ap()
    wproj = nc.dram_tensor("wproj", (CH, C), BF16, kind="ExternalInput").ap()
    maskd = nc.dram_tensor("mask", (128, 128), BF16, kind="ExternalInput").ap()
    out = nc.dram_tensor("out", (TALL, C), BF16, kind="ExternalOutput").ap()

    with tile.TileContext(nc) as tc:
        with (
            tc.tile_pool(name="singles", bufs=1) as singles,
            tc.tile_pool(name="xpool", bufs=3) as xpool,
            tc.tile_pool(name="expp", bufs=7) as expp,
            tc.tile_pool(name="normp", bufs=12) as normp,
            tc.tile_pool(name="ytmp", bufs=2) as ytmpp,
            tc.tile_pool(name="outp", bufs=4) as outp,
            tc.tile_pool(name="ps_mm", bufs=MM_B, space="PSUM") as ps_mm,
            tc.tile_pool(name="ps_st", bufs=ST_B, space="PSUM") as ps_st,
            tc.tile_pool(name="ps_yt", bufs=YT_B, space="PSUM") as ps_yt,
        ):
            # ---- constants / weights in SBUF ----
            # prefetch the first x chunk before the weights so PE starts
            # early; per-c-tile DMAs so the ct=0 matmul can go after ~128KB
            xc0 = xpool.tile([128, NCT, TCH], BF16, tag="xc", name="xc0")
            wq_sb = singles.tile([128, NCT, CH], BF16, tag="wq")
            wk_sb = singles.tile([128, NCT, CH], BF16, tag="wk")
            wv_sb = singles.tile([128, NCT, CH], BF16, tag="wv")
            # first contraction tile of wq + first x slice go down the sync
            # queue so the first matmul can start ~2us in; the bulk of the
            # weights goes down the SWDGE queue, in parallel with the x loads
            # x chunk 0 in two batched DMAs: the shared HWDGE dispatcher
            # costs 625ns per DMA, so a dozen small loads would serialize
            # the cold start.  The first half goes absolutely first so the
            # ct-major q/k matmul stream can start ~3us in
            nc.sync.dma_start(
                xc0[:, 0:4, :],
                xT[0:4, :, 0:TCH].rearrange("ct p m -> p ct m"))
            nc.sync.dma_start(wq_sb[:, 0, :], wq[0])
            nc.sync.dma_start(wk_sb[:, 0, :], wk[0])
            nc.sync.dma_start(
                xc0[:, 4:NCT, :],
                xT[4:NCT, :, 0:TCH].rearrange("ct p m -> p ct m"))
            nc.sync.dma_start(wv_sb[:, 0, :], wv[0])
            nc.gpsimd.dma_start(
                wq_sb[:, 1:NCT, :], wq[1:NCT].rearrange("ct p m -> p ct m"))
            nc.gpsimd.dma_start(
                wk_sb[:, 1:NCT, :], wk[1:NCT].rearrange("ct p m -> p ct m"))
            nc.gpsimd.dma_start(
                wv_sb[:, 1:NCT, :], wv[1:NCT].rearrange("ct p m -> p ct m"))
            bq_sb = singles.tile([CH, 1], F32, tag="bq")
            bk_sb = singles.tile([CH, 1], F32, tag="bk")
            bvr_sb = singles.tile([1, CH], BF16, tag="bvr")
            nc.sync.dma_start(bq_sb[:], bq)
            nc.sync.dma_start(bk_sb[:], bk)
            nc.sync.dma_start(bvr_sb[:], bvr)
            wproj_sb = singles.tile([CH, C], BF16, tag="wpr")
            nc.gpsimd.dma_start(wproj_sb[:], wproj)
            # head-1 rows of wproj at base partition 0: the LAST chunk's
            # projection contracts per-head (split-K) so it never waits for
            # the h1 y repack DMA
            wphi_sb = singles.tile([D, C], BF16, tag="wph")
            nc.gpsimd.dma_start(wphi_sb[:], wproj[D:CH, :])
            mask_sb = singles.tile([128, 128], BF16, tag="mask")
            nc.gpsimd.dma_start(mask_sb[:], maskd)

            # ones rows for the outer-product broadcast matmuls
            ones_sb = singles.tile([1, D], BF16, tag="ones")
            nc.vector.memset(ones_sb[:], 1.0)
            ones128_sb = singles.tile([1, 128], BF16, tag="ones128")
            nc.vector.memset(ones128_sb[:], 1.0)

            # per-chunk activations (fine-grained deps => phases pipeline)
            qT_c = [singles.tile([CH, TCH], BF16, tag=f"qT{i}", name=f"qT{i}")
                    for i in range(NCHUNK)]
            kT_c = [singles.tile([CH, TCH], BF16, tag=f"kT{i}", name=f"kT{i}")
                    for i in range(NCHUNK)]
            # v layout per chunk: [k-part, k-tile-in-chunk, head, 65]
            v_c = [singles.tile([128, 4, HPC, D + 1], BF16, tag=f"v{i}", name=f"v{i}")
                   for i in range(NCHUNK)]
            for i in range(NCHUNK):
                nc.vector.memset(v_c[i][:, :, :, D : D + 1], 1.0)
            y_c = [singles.tile([CH, TCH], BF16, tag=f"y{i}", name=f"y{i}")
                   for i in range(NCHUNK)]

            for _rep in range(repeat):
                # FILLER UNITS: qkv / projection / store work is chopped into
                # ~1-instruction closures on this deque and popped between the
                # attention score/av tiles, so the PE always has independent
                # matmuls to chew on while the Activation engine runs exp
                # (the exp chain is ~40% slower than the dependent PE work)
                # hard units: next chunk's qkv — must fully drain before the
                # chunk that consumes the q/k/v.  soft units: norm finish /
                # projection / stores — can drain any time after emission
                # order is fixed (FIFO within the deque preserves producer ->
                # consumer order)
                hard_units = deque()
                soft_units = deque()

                def units_len():
                    return len(hard_units) + len(soft_units)

                def pop_units(n):
                    for _ in range(n):
                        if hard_units:
                            hard_units.popleft()()
                        elif soft_units:
                            soft_units.popleft()()
                        else:
                            break

                # ---- phase A: qkv projection of one 512-token chunk ----
                def build_qkv_units(chunk, first=False):
                    t0 = chunk * TCH
                    if first:
                        xc = xc0
                    else:
                        xc = xpool.tile([128, NCT, TCH], BF16, tag="xc")
                        # four batched loads (issued NOW, one chunk ahead):
                        # batching bounds the 625ns/DMA HWDGE dispatch cost,
                        # 2-ct granularity keeps the first slices landing
                        # early enough for the first filler matmuls
                        for q4 in range(0, NCT, 2):
                            nc.sync.dma_start(
                                xc[:, q4 : q4 + 2, :],
                                xT[q4 : q4 + 2, :, t0 : t0 + TCH]
                                .rearrange("ct p m -> p ct m"))

                    def chain(w_sb, bias_sb, dst, collect=None):
                        box = {}
                        def mk_mm(ct):
                            def f():
                                if ct == 0:
                                    box["ps"] = ps_mm.tile(
                                        [128, TCH], F32, tag="mm",
                                        name="qkvps")
                                nc.tensor.matmul(
                                    box["ps"][:], w_sb[:, ct, :], xc[:, ct, :],
                                    start=(ct == 0), stop=(ct == NCT - 1),
                                )
                            return f
                        mms = [mk_mm(ct) for ct in range(NCT)]
                        mms.append(lambda: nc.vector.tensor_scalar_add(
                            dst[:], box["ps"][:], bias_sb[:]))
                        if collect is None:
                            hard_units.extend(mms)
                        else:
                            collect.append(mms)
                        return box

                    if first:
                        # cold start: no other work exists to hide the x DMA
                        # latency, so interleave the q/k chains ct-major —
                        # every arriving 128KB x slice immediately feeds two
                        # matmuls instead of one
                        qk = []
                        chain(wq_sb, bq_sb, qT_c[chunk], collect=qk)
                        chain(wk_sb, bk_sb, kT_c[chunk], collect=qk)
                        for uq, uk in zip(*qk):
                            hard_units.append(uq)
                            hard_units.append(uk)
                    else:
                        chain(wq_sb, bq_sb, qT_c[chunk])
                        chain(wk_sb, bk_sb, kT_c[chunk])
                    # v: computed directly in natural [token, channel] layout
                    # (out partitions = tokens), one 128-token group at a
                    # time; the bias lands via a ones outer-product matmul
                    # that seeds the PSUM accumulation, so no activation op
                    # and no transposes
                    def mk_v_group(s):
                        box = {}
                        def pre():
                            box["ps"] = ps_mm.tile([128, CH], F32, tag="mm",
                                                   name="vps")
                            nc.tensor.matmul(
                                box["ps"][:], ones128_sb[:], bvr_sb[:],
                                start=True, stop=False,
                            )
                        us = [pre]
                        def mk_mm(ct):
                            def f():
                                nc.tensor.matmul(
                                    box["ps"][:], xc[:, ct, s * 128 : (s + 1) * 128],
                                    wv_sb[:, ct, :],
                                    start=False, stop=(ct == NCT - 1),
                                )
                            return f
                        for ct in range(NCT):
                            us.append(mk_mm(ct))
                        def cp():
                            nc.vector.tensor_copy(
                                v_c[chunk][:, s, :, 0:D],
                                box["ps"].rearrange("p (h d) -> p h d", h=HPC),
                            )
                        us.append(cp)
                        return us
                    for s in range(TCH // 128):
                        hard_units.extend(mk_v_group(s))

                # ---- phase B+C: attention. The two heads' score/exp/av
                # tiles are interleaved so the PE streams head h1's scores
                # while the Activation engine runs head h0's exp (the exp of
                # one tile is ~40% slower than its dependent PE work); av
                # matmuls trail one tile behind their exp so they never wait.
                lastbox = {}

                def mk_norm_finish(b, c, h, yt_sb, recip_sb, last=False):
                    # broadcast 1/denominator 1 -> 64 partitions with a
                    # single outer-product matmul, then scale y.  Runs as a
                    # deferred unit: by pop time the reciprocal is long done,
                    # so the matmul never stalls the PE wait queue.
                    def f():
                        bc_ps = ps_mm.tile([64, 512], F32, tag="mm",
                                           name="bc_ps")
                        nc.tensor.matmul(
                            bc_ps[:], ones_sb[:], recip_sb[:],
                            start=True, stop=True,
                        )
                        yt_dst = y_c[b * NQC + c]
                        if h == 0:
                            nc.vector.tensor_mul(
                                yt_dst[0:D, :], yt_sb[0:D, :], bc_ps[:])
                        else:
                            yh_sb = ytmpp.tile([D, 512], BF16, tag="yb",
                                               name="yh_sb")
                            nc.vector.tensor_mul(
                                yh_sb[:], yt_sb[0:D, :], bc_ps[:])
                            if last:
                                # last chunk: no repack DMA — the split-K
                                # projection reads this tile directly
                                lastbox["yh1"] = yh_sb
                            else:
                                nc.gpsimd.dma_start(
                                    yt_dst[D : 2 * D, :], yh_sb[:])
                    return f

                def emit_chunk_cells(b, c, last=False):
                    jorder = list(range(4 * c + 1)) + [4 * c + 1, 4 * c + 3,
                                                       4 * c + 2]
                    jlast = jorder[-1]
                    tiles_spec = []
                    fill = 0
                    cur = []
                    for j in jorder:
                        qoff = 0 if j < 4 * c else (j - 4 * c) * 128
                        w = 512 - qoff
                        if cur and fill + w > ST_W:
                            tiles_spec.append((fill, cur))
                            cur = []
                            fill = 0
                        cur.append((j, fill, qoff, w))
                        fill += w
                    tiles_spec.append((fill, cur))
                    ntiles = len(tiles_spec)

                    yt_ps = [ps_yt.tile([D + 1, 512], F32, tag="yt",
                                        name=f"yt{h}") for h in range(HPC)]

                    def emit_avs(ests, cc):
                        for h in range(HPC):
                            for j, off, qoff, w in cc:
                                kt = b * NKT_B + j
                                nc.tensor.matmul(
                                    yt_ps[h][:, qoff:512],
                                    v_c[kt // 4][:, kt % 4, h, :],
                                    ests[h][:, off : off + w],
                                    start=(j == 0), stop=(j == jlast),
                                )

                    # avs trail their exp by TWO tiles: by the time an av
                    # matmul reaches the head of the PE wait queue its est is
                    # long since written, so it never dams up the (in-order,
                    # depth-4) queue in front of ready score/filler matmuls
                    pending = deque()
                    for t, (fill, cc) in enumerate(tiles_spec):
                        ests = []
                        for h in range(HPC):
                            hb = h * D
                            st = ps_st.tile([128, ST_W], F32, tag="st",
                                            name="st")
                            est = expp.tile([128, ST_W], BF16, tag="est",
                                            name="est")
                            for j, off, qoff, w in cc:
                                kTh_j = kT_c[b * NQC + j // 4][
                                    hb : hb + D,
                                    (j % 4) * 128 : (j % 4 + 1) * 128]
                                nc.tensor.matmul(
                                    st[:, off : off + w], kTh_j,
                                    qT_c[b * NQC + c][hb : hb + D, qoff:512],
                                    start=True, stop=True,
                                )
                            nc.scalar.activation(
                                est[:, 0:fill], st[:, 0:fill],
                                mybir.ActivationFunctionType.Exp,
                                scale=1.0 / np.sqrt(D),
                            )
                            for j, off, qoff, w in cc:
                                if j >= 4 * c:  # diagonal: causal mask
                                    # on gpsimd: off the Activation engine so
                                    # exps stream back-to-back
                                    nc.gpsimd.tensor_mul(
                                        est[:, off : off + 128],
                                        est[:, off : off + 128],
                                        mask_sb[:],
                                    )
                            ests.append(est)
                        pending.append((ests, cc))
                        # filler units (next chunk's qkv, previous chunks'
                        # projection/stores) run while this tile's exp cooks.
                        # The last site holds 6 back: they become the next
                        # chunk's first-tile filler (their inputs are a chunk
                        # old, so they never dam the PE wait queue)
                        k = -(-units_len() // (ntiles - t))
                        if t == ntiles - 1 and not last:
                            k = max(0, units_len() - 6)
                        pop_units(k)
                        if len(pending) > 2:
                            emit_avs(*pending.popleft())
                    while pending:
                        emit_avs(*pending.popleft())

                    # denominators: move yt to SBUF (frees the psum slots),
                    # reciprocal now; the dependent broadcast matmul + scale
                    # run later as deferred units
                    for h in range(HPC):
                        yt_sb = normp.tile([D + 1, 512], BF16, tag="nrm",
                                           name=f"yts{h}")
                        if last and h == 0:
                            # final drain: exps are over, parallelize the
                            # serial norm chain across ACT and DVE
                            nc.scalar.activation(
                                yt_sb[:], yt_ps[h][:],
                                mybir.ActivationFunctionType.Identity)
                        else:
                            nc.vector.tensor_copy(yt_sb[:], yt_ps[h][:])
                        recip_sb = normp.tile([1, 512], BF16, tag="nrm",
                                              name=f"rc{h}")
                        with nc.allow_low_precision(
                                reason="softmax weights are bf16"):
                            nc.vector.reciprocal(
                                recip_sb[:], yt_sb[D : D + 1, :])
                        soft_units.append(
                            mk_norm_finish(b, c, h, yt_sb, recip_sb,
                                           last=last))

                out_pend = []

                def mk_out_dma(g, box, last=False):
                    def f():
                        o = out.rearrange("(tt p) m -> tt p m", p=128)
                        for s2 in range(2):
                            # at the very end the sync queue is idle: split
                            # the final drain across both DMA queues
                            eng = nc.sync if (last and s2 == 0) else nc.gpsimd
                            eng.dma_start(
                                o[g * 4 + s2 * 2 : g * 4 + s2 * 2 + 2]
                                .rearrange("tt p m -> p tt m"),
                                box["ot"][:, s2 * 2 : s2 * 2 + 2, :])
                    return f

                def push_proj_units(b, c, last=False):
                    # bf16 partial; bias added on host after the cross-core
                    # sum.  The whole chunk lands in one SBUF tile; its two
                    # wide DMAs are deferred another chunk so the gpsimd
                    # queue never blocks waiting on fresh data
                    g = b * NQC + c
                    box = {}
                    def mk(s4, half):
                        def f():
                            if s4 == 0 and half == 0:
                                box["ot"] = outp.tile(
                                    [128, 4, C], BF16, tag="ot", name="ot")
                            pso = ps_mm.tile([128, 512], F32, tag="mm", name="pso")
                            ts = s4 * 128
                            hs = half * 512
                            if last:
                                # split-K: h0 straight after its norm, h1
                                # from the un-repacked tile — no y DMA
                                nc.tensor.matmul(
                                    pso[:], y_c[g][0:D, ts : ts + 128],
                                    wproj_sb[0:D, hs : hs + 512],
                                    start=True, stop=False,
                                )
                                nc.tensor.matmul(
                                    pso[:],
                                    lastbox["yh1"][:, ts : ts + 128],
                                    wphi_sb[:, hs : hs + 512],
                                    start=False, stop=True,
                                )
                            else:
                                nc.tensor.matmul(
                                    pso[:], y_c[g][:, ts : ts + 128],
                                    wproj_sb[:, hs : hs + 512],
                                    start=True, stop=True,
                                )
                            dst = box["ot"][:, s4, hs : hs + 512]
                            if last and half == 1:
                                # final drain: exp stream is over, use the
                                # idle Activation engine for half the copies
                                nc.scalar.activation(
                                    dst, pso[:],
                                    mybir.ActivationFunctionType.Identity)
                            else:
                                nc.vector.tensor_copy(dst, pso[:])
                        return f
                    def mk_row_dma(s4):
                        # final drain: ship each 128-token row block the
                        # moment its two copies land, alternating DMA queues
                        def f():
                            o = out.rearrange("(tt p) m -> tt p m", p=128)
                            eng = nc.sync if s4 % 2 == 0 else nc.gpsimd
                            eng.dma_start(o[g * 4 + s4], box["ot"][:, s4, :])
                        return f
                    for s4 in range(4):
                        for half in range(2):
                            soft_units.append(mk(s4, half))
                        if last:
                            soft_units.append(mk_row_dma(s4))
                    if out_pend:
                        soft_units.append(out_pend.pop(0))
                    if not last:
                        out_pend.append(mk_out_dma(g, box))

                # ---- interleaved emission ----
                if "qkv" in phases:
                    build_qkv_units(0, first=(_rep == 0))
                    pop_units(units_len())
                if "att" in phases:
                    for g in range(NCHUNK):
                        b, c = divmod(g, NQC)
                        # any units still queued from the previous chunk are
                        # this chunk's qkv holdback: emit them now, before
                        # its cells consume q/k/v
                        pop_units(units_len())
                        if "qkv" in phases and g + 1 < NCHUNK:
                            build_qkv_units(g + 1)
                        emit_chunk_cells(b, c, last=(g == NCHUNK - 1))
                        if "proj" in phases:
                            push_proj_units(b, c, last=(g == NCHUNK - 1))
                    pop_units(units_len())
                    for f in out_pend:
                        f()
                    del out_pend[:]
                elif "qkv" in phases:
                    for g in range(1, NCHUNK):
                        build_qkv_units(g)
                    pop_units(units_len())


    nc.compile()
    return nc


_NC_CACHE = None


def _get_nc():
    global _NC_CACHE
    if _NC_CACHE is None:
        _NC_CACHE = build_nc()
    return _NC_CACHE


def _bf16(a):
    import ml_dtypes
    return np.asarray(a, dtype=np.float32).astype(ml_dtypes.bfloat16)


def make_in_maps(x, Wqkv, bqkv, Wproj, bproj):
    x = np.asarray(x, dtype=np.float32)
    Wqkv = np.asarray(Wqkv, dtype=np.float32)
    bqkv = np.asarray(bqkv, dtype=np.float32)
    Wproj = np.asarray(Wproj, dtype=np.float32)

    x_flat = x.reshape(TALL, C)
    xT = _bf16(np.ascontiguousarray(x_flat.T).reshape(NCT, 128, TALL))
    mask = _bf16(np.triu(np.ones((128, 128), dtype=np.float32)))

    in_maps = []
    for i in range(N_CORES):
        cs = slice(i * CH, (i + 1) * CH)
        in_maps.append({
            "xT": xT,
            "wq": _bf16(np.ascontiguousarray(Wqkv[:, cs]).reshape(NCT, 128, CH)),
            "wk": _bf16(np.ascontiguousarray(Wqkv[:, C + i * CH : C + (i + 1) * CH]
                                             ).reshape(NCT, 128, CH)),
            "wv": _bf16(np.ascontiguousarray(Wqkv[:, 2 * C + i * CH : 2 * C + (i + 1) * CH]
                                             ).reshape(NCT, 128, CH)),
            "bq": np.ascontiguousarray(bqkv[cs]).reshape(CH, 1),
            "bk": np.ascontiguousarray(bqkv[C + i * CH : C + (i + 1) * CH]
                                       ).reshape(CH, 1),
            "bvr": _bf16(bqkv[2 * C + i * CH : 2 * C + (i + 1) * CH]
                         .reshape(1, CH)),
            "wproj": _bf16(np.ascontiguousarray(Wproj[cs, :])),
            "mask": mask,
        })
    return in_maps


def kernel(x, Wqkv, bqkv, Wproj, bproj, _trace=False, _trace_kwargs=None):
    nc = _get_nc()
    in_maps = make_in_maps(x, Wqkv, bqkv, Wproj, bproj)
    res = run_bass_kernel_spmd(
        nc, in_maps, core_ids=list(range(N_CORES)),
        trace=_trace, **(_trace_kwargs or {}),
    )
    acc = res.results[0]["out"].astype(np.float32).copy()
    for c in range(1, N_CORES):
        acc += res.results[c]["out"]
    acc += np.asarray(bproj, dtype=np.float32)[None, :]
    out = acc.reshape(B, T, C)
    if _trace:
        return out, res
    return out
